# revision 4
# baseline (speedup 1.0000x reference)
"""Causal self-attention (B=2, T=2048, D=768, H=12) on 8 TRN2 cores.

Sharding: core r handles batch b=r//4 and head-group g=r%4 (3 heads).
  - qkv projection: tensor-parallel slice of W_qkv (this core's 3 heads).
  - attention: fully local per (b, head).
  - reshard: one AllToAll moves pre-projection attention outputs (O^T,
    feature-major; slab s = [192 rows x 256 tokens] destined to core s)
    so core r ends up with all 768 attention features for token block
    r*256:(r+1)*256 of BOTH batches.
  - proj: y^T[o, t] = sum_f wpT[f, o] otf[f, t] accumulated over 6
    feature chunks in PSUM; f16 y^T out, host transposes.

Schedule: per 512-token block bi, qkv chunk bi interleaved with heads
0/1 attention (S for both heads lands in one [128,2,512] PSUM tile ->
one EXP; the two S matmuls run concurrently on disjoint PE row groups).
Head 2 follows as phase 2 with paired key-tiles sharing one EXP.
Normalization is fused into the PSUM->SBUF eviction (approx reciprocal
+ partition broadcast + multiply-from-PSUM).  The AllToAll is issued
after phase 2; warmup matmuls keep the PE clock (HAM gate) high across
the collective so the proj runs at full speed.

DMA: x split into 18 column-piece DMAs over sync+scalar HWDGE queues
(a single 512KB DMA is ~12us latency-bound); weights on gpsimd SW DGE.
"""

import numpy as np

import concourse.bass as bass
import concourse.bacc as bacc
import concourse.mybir as mybir
import concourse.tile as tile
from concourse.bass_utils import run_bass_kernel_spmd

F32 = mybir.dt.float32
F16 = mybir.dt.float16

B, T, D = 2, 2048, 768
H, DH = 12, 64
NCORES = 8
HPC = H // 4           # heads per core = 3
QK = HPC * DH          # 192 rows of q (or k) per core
KC = D // 128          # 6 contraction chunks
TBLK = T // NCORES     # 256 tokens of proj output per core
NT = T // 128          # 16 key tiles
VW = 65                # v_aug row width per tile (64 + ones col)

EXP_SCALE = 1.0 / np.sqrt(DH)  # 0.125


def _emit(tc, aps):
    nc = tc.nc
    xT, wqkT, wvT, wpT, triu, y = (
        aps["xT"], aps["wqkT"], aps["wvT"], aps["wpT"], aps["triu"], aps["y"])

    ctx_pools = {}

    def pool(name, bufs, space="SBUF"):
        p = tc.tile_pool(name=name, bufs=bufs, space=space)
        ctx_pools[name] = p
        return p.__enter__()

    def close_pool(name):
        ctx_pools.pop(name).__exit__(None, None, None)

    consts = pool("consts", 1)
    qk_sb = pool("qk_sb", 1)
    v_sb = pool("v_sb", 1)
    ot_sb = pool("ot_sb", 1)
    work = pool("work", 3)
    norm = pool("norm", 3)
    dram = pool("dram", 1, space="DRAM")
    xw = pool("xw", 1)

    # ---- input loads ----
    triu_sb = consts.tile([128, 128], F16, tag="triu", name="triu")
    nc.gpsimd.dma_start(triu_sb[:], triu[:, :])

    # x in 18 column pieces: [0:512) quarters first (gates the first
    # m-group), then [512:1280) and [1280:2048) halves of the rest.
    xT_sb = [xw.tile([128, T], F16, tag=f"xT{k}", name=f"xT{k}") for k in range(KC)]
    for cs in (slice(0, 512), slice(512, 1280), slice(1280, 2048)):
        for k in range(KC):
            eng = nc.sync if k % 2 == 0 else nc.scalar
            eng.dma_start(xT_sb[k][:, cs], xT[k * 128:(k + 1) * 128, cs])

    wqk_sb = [consts.tile([128, 2 * QK], F16, tag=f"wqk{k}", name=f"wqk{k}") for k in range(KC)]
    wv_sb = [consts.tile([128, QK], F16, tag=f"wv{k}", name=f"wv{k}") for k in range(KC)]
    for k in range(KC):
        nc.gpsimd.dma_start(wqk_sb[k][:], wqkT[k * 128:(k + 1) * 128, :])
        nc.gpsimd.dma_start(wv_sb[k][:], wvT[k * 128:(k + 1) * 128, :])

    wp_sb = [consts.tile([128, D], F16, tag=f"wp{k}", name=f"wp{k}") for k in range(KC)]
    for k in range(KC):
        nc.gpsimd.dma_start(wp_sb[k][:], wpT[k * 128:(k + 1) * 128, :])

    # ---- PE warmup: ramp the HAM clock gate before real work arrives ----
    ps_warm = pool("ps_warm", 1, space="PSUM")
    warm = ps_warm.tile([128, 128], F32, tag="warm", name="warm")
    for _ in range(60):
        nc.tensor.matmul(warm[:], triu_sb[:], triu_sb[:], start=True, stop=True)
    close_pool("ps_warm")

    ps_qkv = pool("ps_qkv", 1, space="PSUM")
    ps_s = pool("ps_s", 2, space="PSUM")
    ps_o = pool("ps_o", 2, space="PSUM")

    # ---- persistent SBUF tensors ----
    qTp = qk_sb.tile([128, T], F16, tag="qTp", name="qTp")
    kTp = qk_sb.tile([128, T], F16, tag="kTp", name="kTp")
    qT2 = qk_sb.tile([64, T], F16, tag="qT2", name="qT2")
    kT2 = qk_sb.tile([64, T], F16, tag="kT2", name="kT2")

    v_aug = [v_sb.tile([128, NT * VW], F16, tag=f"v{h}", name=f"v{h}") for h in range(HPC)]
    nc.vector.memset(v_aug[0][:], 1.0)
    nc.vector.memset(v_aug[1][:], 1.0)
    nc.gpsimd.memset(v_aug[2][:], 1.0)

    OT = [ot_sb.tile([64, T], F16, tag=f"OT{h}", name=f"OT{h}") for h in range(HPC)]

    a2a_in = dram.tile([NCORES * QK, TBLK], F16, tag="a2a_in", name="a2a_in")
    a2a_out = dram.tile([NCORES * QK, TBLK], F16, tag="a2a_out", name="a2a_out")

    # ---- qkv chunk: m-groups and v tiles interleaved (PSUM banks alternate) ----
    def emit_qkv_m(n, m):
        ns = slice(n * 512, (n + 1) * 512)
        ps = ps_qkv.tile([128, 512], F32, tag="qkps", name="qkps")
        for k in range(KC):
            nc.tensor.matmul(
                ps[:], wqk_sb[k][:, m * 128:(m + 1) * 128], xT_sb[k][:, ns],
                start=(k == 0), stop=(k == KC - 1))
        if m == 0:
            nc.vector.tensor_copy(qTp[:, ns], ps[:])
        elif m == 1:
            nc.vector.tensor_copy(qT2[:, ns], ps[0:64, :])
            nc.vector.tensor_copy(kTp[0:64, ns], ps[64:128, :])
        else:
            nc.vector.tensor_copy(kTp[64:128, ns], ps[0:64, :])
            nc.vector.tensor_copy(kT2[:, ns], ps[64:128, :])

    def emit_qkv_v(tt):
        ps = ps_qkv.tile([128, QK], F32, tag="vps", name="vps")
        for k in range(KC):
            nc.tensor.matmul(
                ps[:], xT_sb[k][:, tt * 128:(tt + 1) * 128], wv_sb[k][:],
                start=(k == 0), stop=(k == KC - 1))
        for h in range(HPC):
            nc.vector.tensor_copy(
                v_aug[h][:, tt * VW:tt * VW + 64], ps[:, h * 64:(h + 1) * 64])

    def emit_qkv_chunk(n):
        emit_qkv_m(n, 0)
        emit_qkv_v(4 * n + 0)
        emit_qkv_m(n, 1)
        emit_qkv_v(4 * n + 1)
        emit_qkv_m(n, 2)
        emit_qkv_v(4 * n + 2)
        emit_qkv_v(4 * n + 3)

    def finish_head(h, bi, o_ps):
        iblk = slice(bi * 512, (bi + 1) * 512)
        stage = norm.tile([1, 512], F32, tag="stage", name="stage")
        nc.vector.tensor_copy(stage[:], o_ps[64:65, :])
        rec = norm.tile([1, 512], F32, tag="rec", name="rec")
        nc.vector.reciprocal_approx_fast(rec[:], stage[:])
        rb = norm.tile([64, 512], F32, tag="rb", name="rb")
        nc.gpsimd.partition_broadcast(rb[:], rec[:])
        nc.vector.tensor_mul(OT[h][:, iblk], o_ps[0:64, :], rb[:])
        # stage this head's rows of each token-block slab
        for s in (2 * bi, 2 * bi + 1):
            nc.sync.dma_start(
                a2a_in[s * QK + h * 64:s * QK + (h + 1) * 64, :],
                OT[h][:, s * TBLK:(s + 1) * TBLK])

    # ---- phase 1: qkv + heads 0/1 ----
    for bi in range(T // 512):
        emit_qkv_chunk(bi)
        o01 = [ps_o.tile([65, 512], F32, tag="o", name=f"o{h}_{bi}") for h in range(2)]
        ntj = 4 * bi + 4
        for tj in range(ntj):
            dtile = tj - 4 * bi
            lo = max(dtile, 0) * 128
            s_ps = ps_s.tile([128, 2, 512], F32, tag="s", name="s")
            e_sb = work.tile([128, 2, 512], F16, tag="e", name="e")
            nc.tensor.matmul(
                s_ps[:, 0:1, lo:512].squeeze(1),
                kTp[0:64, tj * 128:(tj + 1) * 128],
                qTp[0:64, bi * 512 + lo:(bi + 1) * 512],
                start=True, stop=True, tile_position=(0, 0))
            nc.tensor.matmul(
                s_ps[:, 1:2, lo:512].squeeze(1),
                kTp[64:128, tj * 128:(tj + 1) * 128],
                qTp[64:128, bi * 512 + lo:(bi + 1) * 512],
                start=True, stop=True, tile_position=(64, 0))
            nc.scalar.activation(
                e_sb[:, :, lo:512], s_ps[:, :, lo:512],
                mybir.ActivationFunctionType.Exp, scale=EXP_SCALE)
            for h in range(2):
                if dtile >= 0:
                    nc.vector.tensor_mul(
                        e_sb[:, h:h + 1, lo:lo + 128].squeeze(1),
                        e_sb[:, h:h + 1, lo:lo + 128].squeeze(1),
                        triu_sb[:])
                nc.tensor.matmul(
                    o01[h][:, lo:],
                    v_aug[h][:, tj * VW:(tj + 1) * VW],
                    e_sb[:, h:h + 1, lo:512].squeeze(1),
                    start=(tj == 0), stop=(tj == ntj - 1))
        for h in range(2):
            finish_head(h, bi, o01[h])

    # ---- phase 2: head 2, paired key tiles sharing one EXP ----
    for bi in range(T // 512):
        o_c = ps_o.tile([65, 512], F32, tag="o", name=f"oc_{bi}")
        ntj = 4 * bi + 4
        for tj0 in range(0, ntj, 2):
            s_ps = ps_s.tile([128, 2, 512], F32, tag="s", name="s2")
            e_sb = work.tile([128, 2, 512], F16, tag="e", name="e2")
            los = []
            for idx, tj in enumerate((tj0, tj0 + 1)):
                lo = max(tj - 4 * bi, 0) * 128
                los.append(lo)
                nc.tensor.matmul(
                    s_ps[:, idx:idx + 1, lo:512].squeeze(1),
                    kT2[:, tj * 128:(tj + 1) * 128],
                    qT2[:, bi * 512 + lo:(bi + 1) * 512],
                    start=True, stop=True)
            nc.scalar.activation(
                e_sb[:, :, los[0]:512], s_ps[:, :, los[0]:512],
                mybir.ActivationFunctionType.Exp, scale=EXP_SCALE)
            for idx, tj in enumerate((tj0, tj0 + 1)):
                lo = los[idx]
                if tj - 4 * bi >= 0:
                    nc.vector.tensor_mul(
                        e_sb[:, idx:idx + 1, lo:lo + 128].squeeze(1),
                        e_sb[:, idx:idx + 1, lo:lo + 128].squeeze(1),
                        triu_sb[:])
                nc.tensor.matmul(
                    o_c[:, lo:],
                    v_aug[2][:, tj * VW:(tj + 1) * VW],
                    e_sb[:, idx:idx + 1, lo:512].squeeze(1),
                    start=(tj == 0), stop=(tj == ntj - 1))
        finish_head(2, bi, o_c)

    nc.gpsimd.collective_compute(
        "AllToAll",
        mybir.AluOpType.bypass,
        replica_groups=[list(range(NCORES))],
        ins=[a2a_in.opt()],
        outs=[a2a_out.opt()],
    )

    close_pool("ps_o")
    close_pool("ps_s")
    close_pool("ps_qkv")
    ps_y = pool("ps_y", 1, space="PSUM")
    proj_sb = pool("proj_sb", 1)

    # keep the PE clock warm across the collective
    warm2 = ps_y.tile([128, 128], F32, tag="warm2", name="warm2")
    for _ in range(50):
        nc.tensor.matmul(warm2[:], triu_sb[:], triu_sb[:], start=True, stop=True)

    # ---- receive: otf[k] [128, 512] holds feature rows 128k..128k+128,
    # cols 0:256 = batch-0 tokens, 256:512 = batch-1 tokens ----
    otf = [proj_sb.tile([128, 512], F16, tag=f"otf{k}", name=f"otf{k}") for k in range(KC)]
    engs = [nc.sync, nc.scalar]
    ei = 0
    for g in range(4):
        f0 = 192 * g
        pieces = [(f0, 128), (f0 + 128, 64)] if g % 2 == 0 else [(f0, 64), (f0 + 64, 128)]
        for b in range(B):
            for fstart, rows in pieces:
                k, r0 = divmod(fstart, 128)
                src = (4 * b + g) * QK + (fstart - f0)
                engs[ei % 2].dma_start(
                    otf[k][r0:r0 + rows, b * TBLK:(b + 1) * TBLK],
                    a2a_out[src:src + rows, :])
                ei += 1

    ps_yt = [ps_y.tile([128, 512], F32, tag=f"y{yc}", name=f"y{yc}") for yc in range(6)]
    yT_sb = [proj_sb.tile([128, 512], F16, tag=f"yT{yc}", name=f"yT{yc}") for yc in range(6)]
    for k in range(KC):
        for yc in range(6):
            nc.tensor.matmul(
                ps_yt[yc][:],
                wp_sb[k][:, yc * 128:(yc + 1) * 128],
                otf[k][:],
                start=(k == 0), stop=(k == KC - 1))
    for yc in range(6):
        nc.vector.tensor_copy(yT_sb[yc][:], ps_yt[yc][:])
        eng = nc.sync if yc % 2 == 0 else nc.scalar
        eng.dma_start(y[yc * 128:(yc + 1) * 128, :], yT_sb[yc][:])

    for name in reversed(list(ctx_pools)):
        close_pool(name)


_NC_CACHE = {}


def _get_nc():
    if "nc" in _NC_CACHE:
        return _NC_CACHE["nc"]
    nc = bacc.Bacc("TRN2", num_devices=NCORES, debug=False)
    aps = {
        "xT": nc.dram_tensor("xT", [D, T], F16, kind="ExternalInput").ap(),
        "wqkT": nc.dram_tensor("wqkT", [D, 2 * QK], F16, kind="ExternalInput").ap(),
        "wvT": nc.dram_tensor("wvT", [D, QK], F16, kind="ExternalInput").ap(),
        "wpT": nc.dram_tensor("wpT", [D, D], F16, kind="ExternalInput").ap(),
        "triu": nc.dram_tensor("triu", [128, 128], F16, kind="ExternalInput").ap(),
        "y": nc.dram_tensor("y", [D, 2 * TBLK], F16, kind="ExternalOutput").ap(),
    }
    with tile.TileContext(nc, num_cores=NCORES) as tc:
        _emit(tc, aps)
    nc.compile()
    _NC_CACHE["nc"] = nc
    return nc


def make_in_maps(x, W_qkv, W_proj):
    triu = np.triu(np.ones((128, 128), dtype=np.float16))
    wpT = np.ascontiguousarray(np.asarray(W_proj).T).astype(np.float16)
    in_maps = []
    for r in range(NCORES):
        b, g = divmod(r, 4)
        rs = slice(QK * g, QK * (g + 1))
        wq = W_qkv[0:D][rs]
        wk = W_qkv[D:2 * D][rs]
        wv = W_qkv[2 * D:3 * D][rs]
        wqkT = np.ascontiguousarray(np.concatenate([wq, wk], axis=0).T).astype(np.float16)
        wvT = np.ascontiguousarray(wv.T).astype(np.float16)
        xT = np.ascontiguousarray(x[b].T).astype(np.float16)
        in_maps.append({"xT": xT, "wqkT": wqkT, "wvT": wvT,
                        "wpT": wpT, "triu": triu})
    return in_maps


def assemble(results):
    y = np.empty((B, T, D), dtype=np.float32)
    for r in range(NCORES):
        yT = np.asarray(results[r]["y"], dtype=np.float32)  # [768, 512]
        y[0, r * TBLK:(r + 1) * TBLK, :] = yT[:, 0:TBLK].T
        y[1, r * TBLK:(r + 1) * TBLK, :] = yT[:, TBLK:2 * TBLK].T
    return y


def kernel(**inputs):
    x = np.asarray(inputs["x"], dtype=np.float32)
    W_qkv = np.asarray(inputs["W_qkv"], dtype=np.float32)
    W_proj = np.asarray(inputs["W_proj"], dtype=np.float32)
    nc = _get_nc()
    in_maps = make_in_maps(x, W_qkv, W_proj)
    res = run_bass_kernel_spmd(nc, in_maps, core_ids=list(range(NCORES)))
    return assemble(res.results)


# revision 8
# speedup vs baseline: 1.0737x; 1.0737x over previous
"""Causal self-attention (B=2, T=2048, D=768, H=12) on 8 TRN2 cores.

Sharding: core r handles batch b=r//4 and head-group g=r%4 (3 heads).
  - qkv projection: tensor-parallel slice of W_qkv (this core's 3 heads).
  - attention: fully local per (b, head).
  - reshard: two AllToAlls (heads 0/1, then head 2) move pre-projection
    attention outputs (O^T, feature-major) so core r ends up with the full
    768 attention features for token block r*256:(r+1)*256 of BOTH batches.
  - proj: y^T[o, t] accumulated in PSUM across both exchanges (4 + 2
    contraction chunks of 128 features); f16 y^T out, host transposes.

Schedule (one pass, engines pipelined):
  phase 1 (per 512-token block bi): qkv chunk bi interleaved with heads
  0/1 attention for bi; softmax denominator via ones-column of V_aug;
  normalization fused into the PSUM->SBUF eviction using
  reciprocal_approx_fast + gpsimd partition_broadcast.  a2a1 issued after
  bi=3.  phase 2: head 2 (paired key-tiles sharing one EXP) overlapping
  a2a1 flight; a2a2 after; proj chunks for heads 0/1 overlap a2a2; tail =
  proj head-2 chunks + single f32->f16 eviction + output DMA.

DMA queues: x split into 24 quarter-chunks over the sync+scalar HWDGE
queues (one big DMA is latency-bound ~12us); weights on gpsimd's SW DGE.
"""

import numpy as np

import concourse.bass as bass
import concourse.bacc as bacc
import concourse.mybir as mybir
import concourse.tile as tile
from concourse.bass_utils import run_bass_kernel_spmd

F32 = mybir.dt.float32
F16 = mybir.dt.float16

B, T, D = 2, 2048, 768
H, DH = 12, 64
NCORES = 8
HPC = H // 4           # heads per core = 3
QK = HPC * DH          # 192 rows of q (or k) per core
KC = D // 128          # 6 contraction chunks
TBLK = T // NCORES     # 256 tokens of proj output per core
NT = T // 128          # 16 key tiles
VW = 65                # v_aug row width per tile (64 + ones col)

EXP_SCALE = 1.0 / np.sqrt(DH)  # 0.125


def _emit(tc, aps):
    nc = tc.nc
    xT, wqkT, wvT, wp01, wp2, triu, y = (
        aps["xT"], aps["wqkT"], aps["wvT"], aps["wp01"], aps["wp2"],
        aps["triu"], aps["y"])

    ctx_pools = {}

    def pool(name, bufs, space="SBUF"):
        p = tc.tile_pool(name=name, bufs=bufs, space=space)
        ctx_pools[name] = p
        return p.__enter__()

    def close_pool(name):
        ctx_pools.pop(name).__exit__(None, None, None)

    consts = pool("consts", 1)
    qk_sb = pool("qk_sb", 1)
    v_sb = pool("v_sb", 1)
    ot_sb = pool("ot_sb", 1)
    work = pool("work", 3)
    norm = pool("norm", 3)
    dram = pool("dram", 1, space="DRAM")
    xw = pool("xw", 1)

    # ---- input loads ----
    # x: 24 quarter-chunk DMAs (one 512KB DMA is ~12us latency on a single
    # queue); even k on sync, odd k on scalar, first-needed quarters first.
    triu_sb = consts.tile([128, 128], F16, tag="triu", name="triu")
    nc.gpsimd.dma_start(triu_sb[:], triu[:, :])

    xT_sb = [xw.tile([128, T], F16, tag=f"xT{k}", name=f"xT{k}") for k in range(KC)]
    for q in range(4):
        qs = slice(q * 512, (q + 1) * 512)
        for k in range(KC):
            eng = nc.sync if k % 2 == 0 else nc.scalar
            eng.dma_start(xT_sb[k][:, qs], xT[k * 128:(k + 1) * 128, qs])

    wqk_sb = [consts.tile([128, 2 * QK], F16, tag=f"wqk{k}", name=f"wqk{k}") for k in range(KC)]
    wv_sb = [consts.tile([128, QK], F16, tag=f"wv{k}", name=f"wv{k}") for k in range(KC)]
    for k in range(KC):
        nc.gpsimd.dma_start(wqk_sb[k][:], wqkT[k * 128:(k + 1) * 128, :])
        nc.gpsimd.dma_start(wv_sb[k][:], wvT[k * 128:(k + 1) * 128, :])

    # proj weights (needed only after a2a1; loads overlap everything)
    wp01_sb = [consts.tile([128, D], F16, tag=f"wp01_{g}", name=f"wp01_{g}") for g in range(4)]
    wp2_sb = [consts.tile([128, D], F16, tag=f"wp2_{j}", name=f"wp2_{j}") for j in range(2)]
    for g in range(4):
        nc.gpsimd.dma_start(wp01_sb[g][:], wp01[g * 128:(g + 1) * 128, :])
    for j in range(2):
        nc.gpsimd.dma_start(wp2_sb[j][:], wp2[j * 128:(j + 1) * 128, :])

    # ---- PE warmup: ramp the HAM clock gate while input DMAs land ----
    ps_warm = pool("ps_warm", 1, space="PSUM")
    warm = ps_warm.tile([128, 128], F32, tag="warm", name="warm")
    for _ in range(30):
        nc.tensor.matmul(warm[:], triu_sb[:], triu_sb[:], start=True, stop=True)
    close_pool("ps_warm")

    ps_qkv = pool("ps_qkv", 1, space="PSUM")
    ps_s = pool("ps_s", 2, space="PSUM")
    ps_o = pool("ps_o", 2, space="PSUM")

    # ---- persistent SBUF tensors ----
    # q/k for heads 0/1 packed in [128, T] (rows 0-63 h0, 64-127 h1);
    # head 2 in [64, T] tiles.
    qTp = qk_sb.tile([128, T], F16, tag="qTp", name="qTp")
    kTp = qk_sb.tile([128, T], F16, tag="kTp", name="kTp")
    qT2 = qk_sb.tile([64, T], F16, tag="qT2", name="qT2")
    kT2 = qk_sb.tile([64, T], F16, tag="kT2", name="kT2")

    # v_aug[h]: [128 token-rows, NT tiles x (64 feats + ones col)]; the ones
    # columns come from a one-time memset (never overwritten by the casts).
    v_aug = [v_sb.tile([128, NT * VW], F16, tag=f"v{h}", name=f"v{h}") for h in range(HPC)]
    nc.vector.memset(v_aug[0][:], 1.0)
    nc.vector.memset(v_aug[1][:], 1.0)
    nc.gpsimd.memset(v_aug[2][:], 1.0)

    OT = [ot_sb.tile([64, T], F16, tag=f"OT{h}", name=f"OT{h}") for h in range(HPC)]

    a2a1_in = dram.tile([NCORES * 128, TBLK], F16, tag="a2a1_in", name="a2a1_in")
    a2a1_out = dram.tile([NCORES * 128, TBLK], F16, tag="a2a1_out", name="a2a1_out")
    a2a2_in = dram.tile([NCORES * 64, TBLK], F16, tag="a2a2_in", name="a2a2_in")
    a2a2_out = dram.tile([NCORES * 64, TBLK], F16, tag="a2a2_out", name="a2a2_out")

    # ---- qkv: one 512-token chunk (m-groups and v tiles interleaved so the
    # two PSUM banks alternate and the PE never waits on an eviction) ----
    def emit_qkv_m(n, m):
        ns = slice(n * 512, (n + 1) * 512)
        ps = ps_qkv.tile([128, 512], F32, tag="qkps", name="qkps")
        for k in range(KC):
            nc.tensor.matmul(
                ps[:], wqk_sb[k][:, m * 128:(m + 1) * 128], xT_sb[k][:, ns],
                start=(k == 0), stop=(k == KC - 1))
        if m == 0:
            nc.vector.tensor_copy(qTp[:, ns], ps[:])
        elif m == 1:
            nc.vector.tensor_copy(qT2[:, ns], ps[0:64, :])
            nc.vector.tensor_copy(kTp[0:64, ns], ps[64:128, :])
        else:
            nc.vector.tensor_copy(kTp[64:128, ns], ps[0:64, :])
            nc.vector.tensor_copy(kT2[:, ns], ps[64:128, :])

    def emit_qkv_v(tt):
        ps = ps_qkv.tile([128, QK], F32, tag="vps", name="vps")
        for k in range(KC):
            nc.tensor.matmul(
                ps[:], xT_sb[k][:, tt * 128:(tt + 1) * 128], wv_sb[k][:],
                start=(k == 0), stop=(k == KC - 1))
        for h in range(HPC):
            nc.vector.tensor_copy(
                v_aug[h][:, tt * VW:tt * VW + 64], ps[:, h * 64:(h + 1) * 64])

    def emit_qkv_chunk(n):
        emit_qkv_m(n, 0)
        emit_qkv_v(4 * n + 0)
        emit_qkv_m(n, 1)
        emit_qkv_v(4 * n + 1)
        emit_qkv_m(n, 2)
        emit_qkv_v(4 * n + 2)
        emit_qkv_v(4 * n + 3)

    # ---- normalization: den row (o_ps[64]) -> approx reciprocal ->
    # partition broadcast -> fused multiply straight out of PSUM ----
    def finish_head(h, bi, o_ps):
        iblk = slice(bi * 512, (bi + 1) * 512)
        stage = norm.tile([1, 512], F32, tag="stage", name="stage")
        nc.vector.tensor_copy(stage[:], o_ps[64:65, :])
        rec = norm.tile([1, 512], F32, tag="rec", name="rec")
        nc.vector.reciprocal_approx_fast(rec[:], stage[:])
        rb = norm.tile([64, 512], F32, tag="rb", name="rb")
        nc.gpsimd.partition_broadcast(rb[:], rec[:])
        nc.vector.tensor_mul(OT[h][:, iblk], o_ps[0:64, :], rb[:])

    # ---- phase 1: qkv + heads 0/1 attention ----
    for bi in range(T // 512):
        emit_qkv_chunk(bi)
        o01 = [ps_o.tile([65, 512], F32, tag="o", name=f"o{h}_{bi}") for h in range(2)]
        ntj = 4 * bi + 4
        for tj in range(ntj):
            dtile = tj - 4 * bi
            lo = max(dtile, 0) * 128
            s_ps = ps_s.tile([128, 2, 512], F32, tag="s", name="s")
            e_sb = work.tile([128, 2, 512], F16, tag="e", name="e")
            nc.tensor.matmul(
                s_ps[:, 0:1, lo:512].squeeze(1),
                kTp[0:64, tj * 128:(tj + 1) * 128],
                qTp[0:64, bi * 512 + lo:(bi + 1) * 512],
                start=True, stop=True, tile_position=(0, 0))
            nc.tensor.matmul(
                s_ps[:, 1:2, lo:512].squeeze(1),
                kTp[64:128, tj * 128:(tj + 1) * 128],
                qTp[64:128, bi * 512 + lo:(bi + 1) * 512],
                start=True, stop=True, tile_position=(64, 0))
            nc.scalar.activation(
                e_sb[:, :, lo:512], s_ps[:, :, lo:512],
                mybir.ActivationFunctionType.Exp, scale=EXP_SCALE)
            for h in range(2):
                if dtile >= 0:
                    nc.vector.tensor_mul(
                        e_sb[:, h:h + 1, lo:lo + 128].squeeze(1),
                        e_sb[:, h:h + 1, lo:lo + 128].squeeze(1),
                        triu_sb[:])
                nc.tensor.matmul(
                    o01[h][:, lo:],
                    v_aug[h][:, tj * VW:(tj + 1) * VW],
                    e_sb[:, h:h + 1, lo:512].squeeze(1),
                    start=(tj == 0), stop=(tj == ntj - 1))
        for h in range(2):
            finish_head(h, bi, o01[h])
            for s in (2 * bi, 2 * bi + 1):
                nc.sync.dma_start(
                    a2a1_in[s * 128 + h * 64:s * 128 + (h + 1) * 64, :],
                    OT[h][:, s * TBLK:(s + 1) * TBLK])

    nc.gpsimd.collective_compute(
        "AllToAll",
        mybir.AluOpType.bypass,
        replica_groups=[list(range(NCORES))],
        ins=[a2a1_in.opt()],
        outs=[a2a1_out.opt()],
    )

    # ---- phase 2: head 2, paired key tiles sharing one EXP ----
    for bi in range(T // 512):
        o_c = ps_o.tile([65, 512], F32, tag="o", name=f"oc_{bi}")
        ntj = 4 * bi + 4
        for tj0 in range(0, ntj, 2):
            s_ps = ps_s.tile([128, 2, 512], F32, tag="s", name="s2")
            e_sb = work.tile([128, 2, 512], F16, tag="e", name="e2")
            los = []
            for idx, tj in enumerate((tj0, tj0 + 1)):
                lo = max(tj - 4 * bi, 0) * 128
                los.append(lo)
                nc.tensor.matmul(
                    s_ps[:, idx:idx + 1, lo:512].squeeze(1),
                    kT2[:, tj * 128:(tj + 1) * 128],
                    qT2[:, bi * 512 + lo:(bi + 1) * 512],
                    start=True, stop=True)
            # one EXP over both halves from the smaller lo; the gap
            # [lo0, lo1) of half 1 holds exp(stale-psum) and is never read.
            nc.scalar.activation(
                e_sb[:, :, los[0]:512], s_ps[:, :, los[0]:512],
                mybir.ActivationFunctionType.Exp, scale=EXP_SCALE)
            for idx, tj in enumerate((tj0, tj0 + 1)):
                lo = los[idx]
                if tj - 4 * bi >= 0:
                    nc.vector.tensor_mul(
                        e_sb[:, idx:idx + 1, lo:lo + 128].squeeze(1),
                        e_sb[:, idx:idx + 1, lo:lo + 128].squeeze(1),
                        triu_sb[:])
                nc.tensor.matmul(
                    o_c[:, lo:],
                    v_aug[2][:, tj * VW:(tj + 1) * VW],
                    e_sb[:, idx:idx + 1, lo:512].squeeze(1),
                    start=(tj == 0), stop=(tj == ntj - 1))
        finish_head(2, bi, o_c)
        for s in (2 * bi, 2 * bi + 1):
            nc.sync.dma_start(
                a2a2_in[s * 64:(s + 1) * 64, :],
                OT[2][:, s * TBLK:(s + 1) * TBLK])

    nc.gpsimd.collective_compute(
        "AllToAll",
        mybir.AluOpType.bypass,
        replica_groups=[list(range(NCORES))],
        ins=[a2a2_in.opt()],
        outs=[a2a2_out.opt()],
    )

    close_pool("ps_o")
    close_pool("ps_s")
    close_pool("ps_qkv")
    ps_y = pool("ps_y", 1, space="PSUM")
    proj_sb = pool("proj_sb", 1)

    # keep the PE clock warm across the collective wait before proj
    warm2 = ps_y.tile([128, 128], F32, tag="warm2", name="warm2")
    for _ in range(40):
        nc.tensor.matmul(warm2[:], triu_sb[:], triu_sb[:], start=True, stop=True)

    # ---- proj: y^T[o, t] accumulated over 6 feature chunks in PSUM.
    # otf01_g [128, 512]: rows = heads 0/1 of group g, cols = b0|b1 tokens.
    otf01 = [proj_sb.tile([128, 512], F16, tag=f"otf01_{g}", name=f"otf01_{g}")
             for g in range(4)]
    for g in range(4):
        nc.sync.dma_start(
            otf01[g][:, 0:TBLK], a2a1_out[g * 128:(g + 1) * 128, :])
        nc.scalar.dma_start(
            otf01[g][:, TBLK:2 * TBLK], a2a1_out[(4 + g) * 128:(5 + g) * 128, :])

    ps_yt = [ps_y.tile([128, 512], F32, tag=f"y{yc}", name=f"y{yc}") for yc in range(6)]
    for g in range(4):
        for yc in range(6):
            nc.tensor.matmul(
                ps_yt[yc][:],
                wp01_sb[g][:, yc * 128:(yc + 1) * 128],
                otf01[g][:],
                start=(g == 0), stop=False)

    # otf2_j [128, 512]: rows = head 2 of groups 2j/2j+1, cols = b0|b1.
    otf2 = [proj_sb.tile([128, 512], F16, tag=f"otf2_{j}", name=f"otf2_{j}")
            for j in range(2)]
    for j in range(2):
        for half in range(2):
            rows = slice(half * 64, (half + 1) * 64)
            src = (2 * j + half) * 64
            nc.sync.dma_start(
                otf2[j][rows, 0:TBLK], a2a2_out[src:src + 64, :])
            nc.scalar.dma_start(
                otf2[j][rows, TBLK:2 * TBLK],
                a2a2_out[NCORES // 2 * 64 + src:NCORES // 2 * 64 + src + 64, :])

    yT_sb = [proj_sb.tile([128, 512], F16, tag=f"yT{yc}", name=f"yT{yc}") for yc in range(6)]
    for j in range(2):
        for yc in range(6):
            nc.tensor.matmul(
                ps_yt[yc][:],
                wp2_sb[j][:, yc * 128:(yc + 1) * 128],
                otf2[j][:],
                start=False, stop=(j == 1))
    for yc in range(6):
        nc.vector.tensor_copy(yT_sb[yc][:], ps_yt[yc][:])
        eng = nc.sync if yc % 2 == 0 else nc.scalar
        eng.dma_start(y[yc * 128:(yc + 1) * 128, :], yT_sb[yc][:])

    for name in reversed(list(ctx_pools)):
        close_pool(name)


_NC_CACHE = {}


def _get_nc():
    if "nc" in _NC_CACHE:
        return _NC_CACHE["nc"]
    nc = bacc.Bacc("TRN2", num_devices=NCORES, debug=False)
    aps = {
        "xT": nc.dram_tensor("xT", [D, T], F16, kind="ExternalInput").ap(),
        "wqkT": nc.dram_tensor("wqkT", [D, 2 * QK], F16, kind="ExternalInput").ap(),
        "wvT": nc.dram_tensor("wvT", [D, QK], F16, kind="ExternalInput").ap(),
        "wp01": nc.dram_tensor("wp01", [512, D], F16, kind="ExternalInput").ap(),
        "wp2": nc.dram_tensor("wp2", [256, D], F16, kind="ExternalInput").ap(),
        "triu": nc.dram_tensor("triu", [128, 128], F16, kind="ExternalInput").ap(),
        "y": nc.dram_tensor("y", [D, 2 * TBLK], F16, kind="ExternalOutput").ap(),
    }
    with tile.TileContext(nc, num_cores=NCORES) as tc:
        _emit(tc, aps)
    nc.compile()
    _NC_CACHE["nc"] = nc
    return nc


def make_in_maps(x, W_qkv, W_proj):
    triu = np.triu(np.ones((128, 128), dtype=np.float16))
    wpT = np.ascontiguousarray(np.asarray(W_proj).T).astype(np.float16)
    # wp01: feature rows {192g .. 192g+128} (heads 3g, 3g+1); wp2: rows
    # {192g+128 .. 192g+192} (head 3g+2), packed per j = g//2.
    wp01 = np.concatenate([wpT[192 * g:192 * g + 128] for g in range(4)], axis=0)
    wp2 = np.concatenate([wpT[192 * g + 128:192 * g + 192] for g in range(4)], axis=0)
    wp01 = np.ascontiguousarray(wp01)
    wp2 = np.ascontiguousarray(wp2)
    in_maps = []
    for r in range(NCORES):
        b, g = divmod(r, 4)
        rs = slice(QK * g, QK * (g + 1))
        wq = W_qkv[0:D][rs]
        wk = W_qkv[D:2 * D][rs]
        wv = W_qkv[2 * D:3 * D][rs]
        wqkT = np.ascontiguousarray(np.concatenate([wq, wk], axis=0).T).astype(np.float16)
        wvT = np.ascontiguousarray(wv.T).astype(np.float16)
        xT = np.ascontiguousarray(x[b].T).astype(np.float16)
        in_maps.append({"xT": xT, "wqkT": wqkT, "wvT": wvT,
                        "wp01": wp01, "wp2": wp2, "triu": triu})
    return in_maps


def assemble(results):
    y = np.empty((B, T, D), dtype=np.float32)
    for r in range(NCORES):
        yT = np.asarray(results[r]["y"], dtype=np.float32)  # [768, 512]
        y[0, r * TBLK:(r + 1) * TBLK, :] = yT[:, 0:TBLK].T
        y[1, r * TBLK:(r + 1) * TBLK, :] = yT[:, TBLK:2 * TBLK].T
    return y


def kernel(**inputs):
    x = np.asarray(inputs["x"], dtype=np.float32)
    W_qkv = np.asarray(inputs["W_qkv"], dtype=np.float32)
    W_proj = np.asarray(inputs["W_proj"], dtype=np.float32)
    nc = _get_nc()
    in_maps = make_in_maps(x, W_qkv, W_proj)
    res = run_bass_kernel_spmd(nc, in_maps, core_ids=list(range(NCORES)))
    return assemble(res.results)


# revision 13
# speedup vs baseline: 1.1682x; 1.0881x over previous
"""Causal self-attention (B=2, T=2048, D=768, H=12) on 8 TRN2 cores.

Sharding: core r handles batch b=r//4 and head-group g=r%4 (3 heads).
  - qkv projection: tensor-parallel slice of W_qkv (this core's 3 heads).
  - attention: fully local per (b, head).
  - reshard: two AllToAlls (heads 0/1, then head 2) move pre-projection
    attention outputs (O^T, feature-major) so core r ends up with the full
    768 attention features for token block r*256:(r+1)*256 of BOTH batches.
  - proj: y^T[o, t] accumulated in PSUM across both exchanges (4 + 2
    contraction chunks of 128 features); f16 y^T out, host transposes.

Schedule (one pass, engines pipelined):
  phase 1 (per 512-token block bi): qkv chunk bi interleaved with heads
  0/1 attention for bi; softmax denominator via ones-column of V_aug;
  normalization fused into the PSUM->SBUF eviction using
  reciprocal_approx_fast + gpsimd partition_broadcast.  a2a1 issued after
  bi=3.  phase 2: head 2 (paired key-tiles sharing one EXP) overlapping
  a2a1 flight; a2a2 after; proj chunks for heads 0/1 overlap a2a2; tail =
  proj head-2 chunks + single f32->f16 eviction + output DMA.

DMA queues: x split into 24 quarter-chunks over the sync+scalar HWDGE
queues (one big DMA is latency-bound ~12us); weights on gpsimd's SW DGE.
"""

import numpy as np

import concourse.bass as bass
import concourse.bacc as bacc
import concourse.mybir as mybir
import concourse.tile as tile
from concourse.bass_utils import run_bass_kernel_spmd

F32 = mybir.dt.float32
F16 = mybir.dt.float16

B, T, D = 2, 2048, 768
H, DH = 12, 64
NCORES = 8
HPC = H // 4           # heads per core = 3
QK = HPC * DH          # 192 rows of q (or k) per core
KC = D // 128          # 6 contraction chunks
TBLK = T // NCORES     # 256 tokens of proj output per core
NT = T // 128          # 16 key tiles
VW = 65                # v_aug row width per tile (64 + ones col)

EXP_SCALE = 1.0 / np.sqrt(DH)  # 0.125


def _emit(tc, aps):
    nc = tc.nc
    xT, wqkT, wvT, wp01, wp2, triu, y = (
        aps["xT"], aps["wqkT"], aps["wvT"], aps["wp01"], aps["wp2"],
        aps["triu"], aps["y"])

    ctx_pools = {}

    def pool(name, bufs, space="SBUF"):
        p = tc.tile_pool(name=name, bufs=bufs, space=space)
        ctx_pools[name] = p
        return p.__enter__()

    def close_pool(name):
        ctx_pools.pop(name).__exit__(None, None, None)

    consts = pool("consts", 1)
    qk_sb = pool("qk_sb", 1)
    v_sb = pool("v_sb", 1)
    ot_sb = pool("ot_sb", 1)
    work = pool("work", 3)
    norm = pool("norm", 3)
    dram = pool("dram", 1, space="DRAM")
    xw = pool("xw", 1)

    # ---- input loads ----
    # x: 24 quarter-chunk DMAs (one 512KB DMA is ~12us latency on a single
    # queue); even k on sync, odd k on scalar, first-needed quarters first.
    triu_sb = consts.tile([128, 128], F16, tag="triu", name="triu")
    nc.gpsimd.dma_start(triu_sb[:], triu[:, :])

    xT_sb = [xw.tile([128, T], F16, tag=f"xT{k}", name=f"xT{k}") for k in range(KC)]
    for q in range(4):
        qs = slice(q * 512, (q + 1) * 512)
        for k in range(KC):
            eng = nc.sync if k % 2 == 0 else nc.scalar
            eng.dma_start(xT_sb[k][:, qs], xT[k * 128:(k + 1) * 128, qs])

    wqk_sb = [consts.tile([128, 2 * QK], F16, tag=f"wqk{k}", name=f"wqk{k}") for k in range(KC)]
    wv_sb = [consts.tile([128, QK], F16, tag=f"wv{k}", name=f"wv{k}") for k in range(KC)]
    for k in range(KC):
        nc.gpsimd.dma_start(wqk_sb[k][:], wqkT[k * 128:(k + 1) * 128, :])
    for k in range(KC):
        nc.gpsimd.dma_start(wv_sb[k][:], wvT[k * 128:(k + 1) * 128, :])

    # proj weights (needed only after a2a1; loads overlap everything)
    wp01_sb = [consts.tile([128, D], F16, tag=f"wp01_{g}", name=f"wp01_{g}") for g in range(4)]
    wp2_sb = [consts.tile([128, D], F16, tag=f"wp2_{j}", name=f"wp2_{j}") for j in range(2)]
    for g in range(4):
        nc.gpsimd.dma_start(wp01_sb[g][:], wp01[g * 128:(g + 1) * 128, :])
    for j in range(2):
        nc.gpsimd.dma_start(wp2_sb[j][:], wp2[j * 128:(j + 1) * 128, :])

    # ---- PE warmup: ramp the HAM clock gate while input DMAs land ----
    ps_warm = pool("ps_warm", 1, space="PSUM")
    warm = ps_warm.tile([128, 128], F32, tag="warm", name="warm")
    for _ in range(30):
        nc.tensor.matmul(warm[:], triu_sb[:], triu_sb[:], start=True, stop=True)
    close_pool("ps_warm")

    ps_qkv = pool("ps_qkv", 1, space="PSUM")
    ps_s = pool("ps_s", 2, space="PSUM")
    ps_o = pool("ps_o", 2, space="PSUM")

    # ---- persistent SBUF tensors ----
    # q/k for heads 0/1 packed in [128, T] (rows 0-63 h0, 64-127 h1);
    # head 2 in [64, T] tiles.
    qTp = qk_sb.tile([128, T], F16, tag="qTp", name="qTp")
    kTp = qk_sb.tile([128, T], F16, tag="kTp", name="kTp")
    qT2 = qk_sb.tile([64, T], F16, tag="qT2", name="qT2")
    kT2 = qk_sb.tile([64, T], F16, tag="kT2", name="kT2")

    # v_all[:, h, tile*65 : tile*65+65]: [128 token-rows, 64 feats + ones
    # col] per (head, key tile); ones columns come from a one-time memset
    # (never overwritten by the casts).
    v_all = v_sb.tile([128, HPC, NT * VW], F16, tag="v_all", name="v_all")
    nc.vector.memset(v_all[:, 0:2, :], 1.0)
    nc.gpsimd.memset(v_all[:, 2:3, :], 1.0)

    OT = [ot_sb.tile([64, T], F16, tag=f"OT{h}", name=f"OT{h}") for h in range(HPC)]

    a2a1_in = dram.tile([NCORES * 128, TBLK], F16, tag="a2a1_in", name="a2a1_in")
    a2a1_out = dram.tile([NCORES * 128, TBLK], F16, tag="a2a1_out", name="a2a1_out")
    a2a2_in = dram.tile([NCORES * 64, TBLK], F16, tag="a2a2_in", name="a2a2_in")
    a2a2_out = dram.tile([NCORES * 64, TBLK], F16, tag="a2a2_out", name="a2a2_out")

    # ---- qkv: one 512-token chunk (m-groups and v tiles interleaved so the
    # two PSUM banks alternate and the PE never waits on an eviction) ----
    def emit_qkv_m(n, m):
        ns = slice(n * 512, (n + 1) * 512)
        ps = ps_qkv.tile([128, 512], F32, tag="qkps", name="qkps")
        for k in range(KC):
            nc.tensor.matmul(
                ps[:], wqk_sb[k][:, m * 128:(m + 1) * 128], xT_sb[k][:, ns],
                start=(k == 0), stop=(k == KC - 1))
        if m == 0:
            nc.vector.tensor_copy(qTp[:, ns], ps[:])
        elif m == 1:
            nc.vector.tensor_copy(qT2[:, ns], ps[0:64, :])
            nc.vector.tensor_copy(kTp[0:64, ns], ps[64:128, :])
        else:
            nc.vector.tensor_copy(kTp[64:128, ns], ps[0:64, :])
            nc.vector.tensor_copy(kT2[:, ns], ps[64:128, :])

    def emit_qkv_v(tt):
        ps = ps_qkv.tile([128, HPC, 64], F32, tag="vps", name="vps")
        for k in range(KC):
            nc.tensor.matmul(
                ps[:], xT_sb[k][:, tt * 128:(tt + 1) * 128], wv_sb[k][:],
                start=(k == 0), stop=(k == KC - 1))
        nc.vector.tensor_copy(
            v_all[:, :, tt * VW:tt * VW + 64], ps[:])

    def qkv_subs(n):
        """Chunk n's qkv emission as 7 thunks, interleaved between the
        previous block's attention tiles so the PE fills EXP-bound gaps."""
        return [
            lambda n=n: emit_qkv_m(n, 0),
            lambda n=n: emit_qkv_v(4 * n + 0),
            lambda n=n: emit_qkv_m(n, 1),
            lambda n=n: emit_qkv_v(4 * n + 1),
            lambda n=n: emit_qkv_m(n, 2),
            lambda n=n: emit_qkv_v(4 * n + 2),
            lambda n=n: emit_qkv_v(4 * n + 3),
        ]

    def emit_qkv_chunk(n):
        for sub in qkv_subs(n):
            sub()

    # ---- normalization: den row (o_ps[64]) -> approx reciprocal ->
    # partition broadcast -> fused multiply straight out of PSUM ----
    def finish_head(h, bi, o_ps):
        iblk = slice(bi * 512, (bi + 1) * 512)
        stage = norm.tile([1, 512], F32, tag="stage", name="stage")
        nc.vector.tensor_copy(stage[:], o_ps[64:65, :])
        rec = norm.tile([1, 512], F32, tag="rec", name="rec")
        nc.vector.reciprocal_approx_fast(rec[:], stage[:])
        rb = norm.tile([64, 512], F32, tag="rb", name="rb")
        nc.gpsimd.partition_broadcast(rb[:], rec[:])
        nc.vector.tensor_mul(OT[h][:, iblk], o_ps[0:64, :], rb[:])

    # ---- phase 1: heads 0/1 attention, next chunk's qkv interleaved ----
    emit_qkv_chunk(0)
    for bi in range(T // 512):
        subs = qkv_subs(bi + 1) if bi + 1 < T // 512 else []
        si = 0
        o01 = [ps_o.tile([65, 512], F32, tag="o", name=f"o{h}_{bi}") for h in range(2)]
        ntj = 4 * bi + 4
        for tj in range(ntj):
            dtile = tj - 4 * bi
            lo = max(dtile, 0) * 128
            s_ps = ps_s.tile([128, 2, 512], F32, tag="s", name="s")
            e_sb = work.tile([128, 2, 512], F16, tag="e", name="e")
            nc.tensor.matmul(
                s_ps[:, 0:1, lo:512].squeeze(1),
                kTp[0:64, tj * 128:(tj + 1) * 128],
                qTp[0:64, bi * 512 + lo:(bi + 1) * 512],
                start=True, stop=True, tile_position=(0, 0))
            nc.tensor.matmul(
                s_ps[:, 1:2, lo:512].squeeze(1),
                kTp[64:128, tj * 128:(tj + 1) * 128],
                qTp[64:128, bi * 512 + lo:(bi + 1) * 512],
                start=True, stop=True, tile_position=(64, 0))
            nc.scalar.activation(
                e_sb[:, :, lo:512], s_ps[:, :, lo:512],
                mybir.ActivationFunctionType.Exp, scale=EXP_SCALE)
            for h in range(2):
                if dtile >= 0:
                    nc.vector.tensor_mul(
                        e_sb[:, h:h + 1, lo:lo + 128].squeeze(1),
                        e_sb[:, h:h + 1, lo:lo + 128].squeeze(1),
                        triu_sb[:])
                nc.tensor.matmul(
                    o01[h][:, lo:],
                    v_all[:, h:h + 1, tj * VW:(tj + 1) * VW].squeeze(1),
                    e_sb[:, h:h + 1, lo:512].squeeze(1),
                    start=(tj == 0), stop=(tj == ntj - 1))
            want = len(subs) * (tj + 1) // ntj
            while si < want:
                subs[si]()
                si += 1
        for h in range(2):
            finish_head(h, bi, o01[h])
            for s in (2 * bi, 2 * bi + 1):
                nc.sync.dma_start(
                    a2a1_in[s * 128 + h * 64:s * 128 + (h + 1) * 64, :],
                    OT[h][:, s * TBLK:(s + 1) * TBLK])

    nc.gpsimd.collective_compute(
        "AllToAll",
        mybir.AluOpType.bypass,
        replica_groups=[list(range(NCORES))],
        ins=[a2a1_in.opt()],
        outs=[a2a1_out.opt()],
    )

    # ---- phase 2: head 2, paired key tiles sharing one EXP ----
    for bi in range(T // 512):
        o_c = ps_o.tile([65, 512], F32, tag="o", name=f"oc_{bi}")
        ntj = 4 * bi + 4
        for tj0 in range(0, ntj, 2):
            s_ps = ps_s.tile([128, 2, 512], F32, tag="s", name="s2")
            e_sb = work.tile([128, 2, 512], F16, tag="e", name="e2")
            los = []
            for idx, tj in enumerate((tj0, tj0 + 1)):
                lo = max(tj - 4 * bi, 0) * 128
                los.append(lo)
                nc.tensor.matmul(
                    s_ps[:, idx:idx + 1, lo:512].squeeze(1),
                    kT2[:, tj * 128:(tj + 1) * 128],
                    qT2[:, bi * 512 + lo:(bi + 1) * 512],
                    start=True, stop=True)
            # one EXP over both halves from the smaller lo; the gap
            # [lo0, lo1) of half 1 holds exp(stale-psum) and is never read.
            nc.scalar.activation(
                e_sb[:, :, los[0]:512], s_ps[:, :, los[0]:512],
                mybir.ActivationFunctionType.Exp, scale=EXP_SCALE)
            for idx, tj in enumerate((tj0, tj0 + 1)):
                lo = los[idx]
                if tj - 4 * bi >= 0:
                    nc.vector.tensor_mul(
                        e_sb[:, idx:idx + 1, lo:lo + 128].squeeze(1),
                        e_sb[:, idx:idx + 1, lo:lo + 128].squeeze(1),
                        triu_sb[:])
                nc.tensor.matmul(
                    o_c[:, lo:],
                    v_all[:, 2:3, tj * VW:(tj + 1) * VW].squeeze(1),
                    e_sb[:, idx:idx + 1, lo:512].squeeze(1),
                    start=(tj == 0), stop=(tj == ntj - 1))
        finish_head(2, bi, o_c)
        for s in (2 * bi, 2 * bi + 1):
            nc.sync.dma_start(
                a2a2_in[s * 64:(s + 1) * 64, :],
                OT[2][:, s * TBLK:(s + 1) * TBLK])

    nc.gpsimd.collective_compute(
        "AllToAll",
        mybir.AluOpType.bypass,
        replica_groups=[list(range(NCORES))],
        ins=[a2a2_in.opt()],
        outs=[a2a2_out.opt()],
    )

    close_pool("ps_o")
    close_pool("ps_s")
    close_pool("ps_qkv")
    ps_y = pool("ps_y", 1, space="PSUM")
    proj_sb = pool("proj_sb", 1)

    # keep the PE clock warm across the collective wait before proj
    warm2 = ps_y.tile([128, 128], F32, tag="warm2", name="warm2")
    for _ in range(40):
        nc.tensor.matmul(warm2[:], triu_sb[:], triu_sb[:], start=True, stop=True)

    # ---- proj: y^T[o, t] accumulated over 6 feature chunks in PSUM.
    # otf01_g [128, 512]: rows = heads 0/1 of group g, cols = b0|b1 tokens.
    otf01 = [proj_sb.tile([128, 512], F16, tag=f"otf01_{g}", name=f"otf01_{g}")
             for g in range(4)]
    for g in range(4):
        nc.sync.dma_start(
            otf01[g][:, 0:TBLK], a2a1_out[g * 128:(g + 1) * 128, :])
        nc.scalar.dma_start(
            otf01[g][:, TBLK:2 * TBLK], a2a1_out[(4 + g) * 128:(5 + g) * 128, :])

    ps_yt = [ps_y.tile([128, 512], F32, tag=f"y{yc}", name=f"y{yc}") for yc in range(6)]
    for g in range(4):
        for yc in range(6):
            nc.tensor.matmul(
                ps_yt[yc][:],
                wp01_sb[g][:, yc * 128:(yc + 1) * 128],
                otf01[g][:],
                start=(g == 0), stop=False)

    # otf2_j [128, 512]: rows = head 2 of groups 2j/2j+1, cols = b0|b1.
    otf2 = [proj_sb.tile([128, 512], F16, tag=f"otf2_{j}", name=f"otf2_{j}")
            for j in range(2)]
    for j in range(2):
        for half in range(2):
            rows = slice(half * 64, (half + 1) * 64)
            src = (2 * j + half) * 64
            nc.sync.dma_start(
                otf2[j][rows, 0:TBLK], a2a2_out[src:src + 64, :])
            nc.scalar.dma_start(
                otf2[j][rows, TBLK:2 * TBLK],
                a2a2_out[NCORES // 2 * 64 + src:NCORES // 2 * 64 + src + 64, :])

    yT_sb = [proj_sb.tile([128, 512], F16, tag=f"yT{yc}", name=f"yT{yc}") for yc in range(6)]
    for j in range(2):
        for yc in range(6):
            nc.tensor.matmul(
                ps_yt[yc][:],
                wp2_sb[j][:, yc * 128:(yc + 1) * 128],
                otf2[j][:],
                start=False, stop=(j == 1))
    for yc in range(6):
        nc.vector.tensor_copy(yT_sb[yc][:], ps_yt[yc][:])
        eng = nc.sync if yc % 2 == 0 else nc.scalar
        eng.dma_start(y[yc * 128:(yc + 1) * 128, :], yT_sb[yc][:])

    for name in reversed(list(ctx_pools)):
        close_pool(name)


_NC_CACHE = {}


def _get_nc():
    if "nc" in _NC_CACHE:
        return _NC_CACHE["nc"]
    nc = bacc.Bacc("TRN2", num_devices=NCORES, debug=False)
    aps = {
        "xT": nc.dram_tensor("xT", [D, T], F16, kind="ExternalInput").ap(),
        "wqkT": nc.dram_tensor("wqkT", [D, 2 * QK], F16, kind="ExternalInput").ap(),
        "wvT": nc.dram_tensor("wvT", [D, QK], F16, kind="ExternalInput").ap(),
        "wp01": nc.dram_tensor("wp01", [512, D], F16, kind="ExternalInput").ap(),
        "wp2": nc.dram_tensor("wp2", [256, D], F16, kind="ExternalInput").ap(),
        "triu": nc.dram_tensor("triu", [128, 128], F16, kind="ExternalInput").ap(),
        "y": nc.dram_tensor("y", [D, 2 * TBLK], F16, kind="ExternalOutput").ap(),
    }
    with tile.TileContext(nc, num_cores=NCORES) as tc:
        _emit(tc, aps)
    nc.compile()
    _NC_CACHE["nc"] = nc
    return nc


def make_in_maps(x, W_qkv, W_proj):
    triu = np.triu(np.ones((128, 128), dtype=np.float16))
    wpT = np.ascontiguousarray(np.asarray(W_proj).T).astype(np.float16)
    # wp01: feature rows {192g .. 192g+128} (heads 3g, 3g+1); wp2: rows
    # {192g+128 .. 192g+192} (head 3g+2), packed per j = g//2.
    wp01 = np.concatenate([wpT[192 * g:192 * g + 128] for g in range(4)], axis=0)
    wp2 = np.concatenate([wpT[192 * g + 128:192 * g + 192] for g in range(4)], axis=0)
    wp01 = np.ascontiguousarray(wp01)
    wp2 = np.ascontiguousarray(wp2)
    in_maps = []
    for r in range(NCORES):
        b, g = divmod(r, 4)
        rs = slice(QK * g, QK * (g + 1))
        wq = W_qkv[0:D][rs]
        wk = W_qkv[D:2 * D][rs]
        wv = W_qkv[2 * D:3 * D][rs]
        wqkT = np.ascontiguousarray(np.concatenate([wq, wk], axis=0).T).astype(np.float16)
        wvT = np.ascontiguousarray(wv.T).astype(np.float16)
        xT = np.ascontiguousarray(x[b].T).astype(np.float16)
        in_maps.append({"xT": xT, "wqkT": wqkT, "wvT": wvT,
                        "wp01": wp01, "wp2": wp2, "triu": triu})
    return in_maps


def assemble(results):
    y = np.empty((B, T, D), dtype=np.float32)
    for r in range(NCORES):
        yT = np.asarray(results[r]["y"], dtype=np.float32)  # [768, 512]
        y[0, r * TBLK:(r + 1) * TBLK, :] = yT[:, 0:TBLK].T
        y[1, r * TBLK:(r + 1) * TBLK, :] = yT[:, TBLK:2 * TBLK].T
    return y


def kernel(**inputs):
    x = np.asarray(inputs["x"], dtype=np.float32)
    W_qkv = np.asarray(inputs["W_qkv"], dtype=np.float32)
    W_proj = np.asarray(inputs["W_proj"], dtype=np.float32)
    nc = _get_nc()
    in_maps = make_in_maps(x, W_qkv, W_proj)
    res = run_bass_kernel_spmd(nc, in_maps, core_ids=list(range(NCORES)))
    return assemble(res.results)
